# revision 1
# baseline (speedup 1.0000x reference)
"""AttentionPooling (segment softmax-pool) Trainium2 kernel.

out[s,:] = sum_n 1[idx[n]==s] * gnorm[n] * (x[n,:] @ msg_w + msg_b)
  gnorm[n] = w[n]^p * exp(gate[n]) / (denom[seg] + eps)   (max-sub skipped:
  mathematically identical after normalization, logits are O(5))

Restructured so the big matmul contracts rows via a one-hot:
  A[s,d]   = sum_n G[n,s] * x[n,d],  denom[s] = sum_n G[n,s]   (ones col)
  out[s,:] = (A[s,:] @ msg_w) / (denom+eps) + (denom/(denom+eps)) * msg_b
where G[n,s] = 1[idx[n]==s] * g[n] is built per 128-row tile with one fused
DVE tensor_scalar(is_equal, mult) against an iota row.

Sharding: index is sorted; host assigns 2048 contiguous segments per core,
16 windows x 128 segments, rows of each window padded to 66*128 = 8448.

Engine assignment (v2): PE = A-matmul + phase2; DVE = G-build, logit reduce,
small ops, phase2 copies; GPSIMD = logit multiply; ACT = exp only (ln hoisted
to one pre-pass) so its LUT never reloads.
"""

import os
import sys
import numpy as np

for _p in ("/opt/trn_rl_repo", "/root/.axon_site/_ro/trn_rl_repo"):
    if os.path.isdir(_p) and _p not in sys.path:
        sys.path.insert(0, _p)

P = 128
S = 16384
D = 128
NCORES = 8
WIN = 64                       # segments per PSUM window
NWIN = S // WIN                # 128 global windows
NWIN_CORE = NWIN // NCORES     # 16 per core
TPW = 34                       # 128-row tiles per window (padded)
GROUP = 17                     # tiles per DMA/logit super-group
GPW = TPW // GROUP             # 6 groups per window
NT = NWIN_CORE * TPW           # 1056 tiles per core
NG = NT // GROUP               # 96 groups per core
ROWS_CORE = NT * P             # 135168 padded rows per core
EPS = 1e-10

IOTA_BF16 = False              # bf16 iota regressed G-build (487 vs 266 ns)
MULT_ON_GPSIMD = False         # gpsimd streaming halves DVE via shared SBUF port
U8_MASK = True                 # host-built u8 one-hot mask kills the is_equal
G_ON_ACT_MOD = 5               # j%5 < 3 -> G-build on ACT (60%); ACT Copy+scale = g*mask
GBUILD_ON_GPSIMD = False       # gpsimd TS measured 2268ns/tile - keep on DVE
ACT_ACCUM_REDUCE = False       # 3D group reduce on DVE hits 2x mode (72ns/tile)

LAST_EXEC_NS = None
LAST_RESULTS = None

_module_cache = {}


def _build_module():
    if "nc" in _module_cache:
        return _module_cache["nc"]

    import concourse.bass as bass  # noqa: F401
    import concourse.tile as tile
    from concourse import bacc, mybir
    from concourse.masks import make_identity

    f32 = mybir.dt.float32
    bf16 = mybir.dt.bfloat16
    iota_dt = bf16 if IOTA_BF16 else f32
    AX = mybir.AxisListType
    ALU = mybir.AluOpType
    ACTF = mybir.ActivationFunctionType

    nc = bacc.Bacc(
        "TRN2",
        target_bir_lowering=False,
        debug=False,
        enable_asserts=True,
        num_devices=NCORES,
    )

    xp = nc.dram_tensor("xp", [NG * P, GROUP * (D + 1)], f32, kind="ExternalInput")
    maskg = nc.dram_tensor(
        "maskg", [NG * P, GROUP * WIN], mybir.dt.uint8, kind="ExternalInput"
    )
    wall = nc.dram_tensor("wall", [P, NT], f32, kind="ExternalInput")
    gwrep = nc.dram_tensor("gwrep", [P, GROUP * D], f32, kind="ExternalInput")
    msgw = nc.dram_tensor("msgw", [D, D], f32, kind="ExternalInput")
    msgbrep = nc.dram_tensor("msgbrep", [P, D], f32, kind="ExternalInput")
    gatebrep = nc.dram_tensor("gatebrep", [P, 1], f32, kind="ExternalInput")
    prep = nc.dram_tensor("prep", [P, 1], f32, kind="ExternalInput")
    out = nc.dram_tensor("out", [NWIN_CORE * WIN, D], f32, kind="ExternalOutput")

    with tile.TileContext(nc) as tc:
        from contextlib import ExitStack

        with ExitStack() as ctx:
            const_pool = ctx.enter_context(tc.tile_pool(name="const", bufs=1))
            xs_pool = ctx.enter_context(tc.tile_pool(name="xs", bufs=10))
            grp_pool = ctx.enter_context(tc.tile_pool(name="grp", bufs=6))
            g_pool = ctx.enter_context(tc.tile_pool(name="gm", bufs=10))
            psA_pool = ctx.enter_context(tc.tile_pool(name="psA", bufs=4, space="PSUM"))
            ps2_pool = ctx.enter_context(tc.tile_pool(name="ps2", bufs=2, space="PSUM"))
            ph2_pool = ctx.enter_context(tc.tile_pool(name="ph2", bufs=3))

            gw_t = const_pool.tile([P, GROUP * D], f32)
            nc.sync.dma_start(gw_t[:], gwrep[:, :])
            msgw_t = const_pool.tile([D, D], f32)
            nc.sync.dma_start(msgw_t[:], msgw[:, :])
            msgb_t = const_pool.tile([P, D], f32)
            nc.sync.dma_start(msgb_t[:], msgbrep[:, :])
            gateb_t = const_pool.tile([P, 1], f32)
            nc.sync.dma_start(gateb_t[:], gatebrep[:, :])
            p_t = const_pool.tile([P, 1], f32)
            nc.sync.dma_start(p_t[:], prep[:, :])
            ident = const_pool.tile([P, P], f32)
            make_identity(nc, ident[:])

            # hoisted: p*ln(w) for every tile in two ops
            w_t = const_pool.tile([P, NT], f32)
            nc.sync.dma_start(w_t[:], wall[:, :])
            plw_t = const_pool.tile([P, NT], f32)
            nc.scalar.activation(out=plw_t[:], in_=w_t[:], func=ACTF.Ln)
            nc.vector.tensor_scalar_mul(plw_t[:], plw_t[:], p_t[:, 0:1])

            gw3 = gw_t[:].rearrange("p (t d) -> p t d", d=D)

            # software pipeline: emit group g+1's logit chain before group g's
            # G-builds so exp(g+1) lands ahead of the G(g) ops in ACT's stream
            chains = {}

            def emit_chain(g):
                xs = xs_pool.tile([P, GROUP * (D + 1)], f32, tag="xs", name=f"xs{g}")
                nc.sync.dma_start(xs[:], xp[g * P : (g + 1) * P, :])
                xs3 = xs[:].rearrange("p (t d) -> p t d", d=D + 1)
                mk = xs_pool.tile(
                    [P, GROUP * WIN], mybir.dt.uint8, tag="mk", name=f"mk{g}"
                )
                nc.sync.dma_start(mk[:], maskg[g * P : (g + 1) * P, :])
                xw = grp_pool.tile([P, GROUP * D], f32, tag="xw", name=f"xw{g}")
                xw3 = xw[:].rearrange("p (t d) -> p t d", d=D)
                nc.vector.tensor_tensor(
                    out=xw3, in0=xs3[:, :, 0:D], in1=gw3, op=ALU.mult
                )
                logit = grp_pool.tile([P, GROUP], f32, tag="logit", name=f"lg{g}")
                nc.vector.reduce_sum(out=logit[:], in_=xw3, axis=AX.X)
                logit2 = grp_pool.tile([P, GROUP], f32, tag="logit2", name=f"l2{g}")
                nc.vector.tensor_add(
                    logit2[:], logit[:], plw_t[:, g * GROUP : (g + 1) * GROUP]
                )
                gex = grp_pool.tile([P, GROUP], f32, tag="gex", name=f"gx{g}")
                nc.scalar.activation(
                    out=gex[:], in_=logit2[:], func=ACTF.Exp, bias=gateb_t[:, 0:1]
                )
                chains[g] = (xs3, mk, gex)

            def emit_gmm(g, psA):
                xs3, mk, gex = chains.pop(g)
                gi = g % GPW
                for j in range(GROUP):
                    t_in_win = gi * GROUP + j
                    t_glob = g * GROUP + j
                    G = g_pool.tile([P, WIN], f32, tag="G", name=f"G{t_glob}")
                    if t_glob % 4 < 3:
                        nc.scalar.activation(
                            out=G[:],
                            in_=mk[:, j * WIN : (j + 1) * WIN],
                            func=ACTF.Copy,
                            scale=gex[:, j : j + 1],
                        )
                    else:
                        nc.vector.tensor_scalar(
                            out=G[:],
                            in0=mk[:, j * WIN : (j + 1) * WIN],
                            scalar1=gex[:, j : j + 1],
                            scalar2=None,
                            op0=ALU.mult,
                        )
                    nc.tensor.matmul(
                        out=psA[:],
                        lhsT=G[:],
                        rhs=xs3[:, j, :],
                        start=(t_in_win == 0),
                        stop=(t_in_win == TPW - 1),
                    )

            def emit_phase2(w, psA):
                sbA = ph2_pool.tile([WIN, D + 1], f32, tag="sbA", name=f"sbA{w}")
                nc.vector.tensor_copy(sbA[:], psA[:])
                deno = ph2_pool.tile([WIN, 1], f32, tag="deno", name=f"dn{w}")
                nc.vector.tensor_scalar_add(deno[:], sbA[:, D : D + 1], EPS)
                rcp = ph2_pool.tile([WIN, 1], f32, tag="rcp", name=f"rc{w}")
                nc.vector.reciprocal(out=rcp[:], in_=deno[:])
                coef = ph2_pool.tile([WIN, 1], f32, tag="coef", name=f"cf{w}")
                nc.vector.tensor_tensor(
                    out=coef[:], in0=sbA[:, D : D + 1], in1=rcp[:], op=ALU.mult
                )
                psAT = ps2_pool.tile([P, WIN], f32, tag="AT", name=f"AT{w}")
                nc.tensor.transpose(
                    out=psAT[:], in_=sbA[:, 0:D], identity=ident[:WIN, :WIN]
                )
                sbAT = ph2_pool.tile([P, WIN], f32, tag="sbAT", name=f"sT{w}")
                nc.vector.tensor_copy(sbAT[:], psAT[:])
                ps2 = ps2_pool.tile([WIN, D], f32, tag="out2", name=f"o2{w}")
                nc.tensor.matmul(
                    out=ps2[:], lhsT=sbAT[:], rhs=msgw_t[:], start=True, stop=True
                )
                outsb = ph2_pool.tile([WIN, D], f32, tag="outsb", name=f"ou{w}")
                nc.scalar.activation(
                    out=outsb[:], in_=ps2[:], func=ACTF.Copy, scale=rcp[:, 0:1]
                )
                bterm = ph2_pool.tile([WIN, D], f32, tag="bterm", name=f"bt{w}")
                nc.scalar.activation(
                    out=bterm[:], in_=msgb_t[:WIN, :], func=ACTF.Copy,
                    scale=coef[:, 0:1],
                )
                ofin = ph2_pool.tile([WIN, D], f32, tag="ofin", name=f"of{w}")
                nc.vector.tensor_add(ofin[:], outsb[:], bterm[:])
                nc.sync.dma_start(out[w * WIN : (w + 1) * WIN, :], ofin[:])

            psA_tiles = {}
            for g in range(NG):
                emit_chain(g)
                w = g // GPW
                if g % GPW == 0:
                    psA_tiles[w] = psA_pool.tile(
                        [WIN, D + 1], f32, tag="psA", name=f"psA{w}"
                    )
                emit_gmm(g, psA_tiles[w])
                if g % GPW == GPW - 1:
                    emit_phase2(w, psA_tiles.pop(w))

    nc.compile()
    _module_cache["nc"] = nc
    return nc


def _shard_inputs(x, idx, w):
    """Pad + reorder host arrays into the per-core device layouts."""
    n = idx.shape[0]
    bounds = np.searchsorted(idx, np.arange(0, S + 1, WIN)).astype(np.int64)
    counts = np.diff(bounds)
    if counts.max() > TPW * P:
        raise RuntimeError(f"window overflow: {counts.max()} > {TPW * P}")

    dest = np.arange(n, dtype=np.int64) + np.repeat(
        np.arange(NWIN, dtype=np.int64) * (TPW * P) - bounds[:-1], counts
    )

    xpad = np.zeros((NCORES * ROWS_CORE, D + 1), dtype=np.float32)
    xpad[:, D] = 1.0
    xpad[dest, 0:D] = x
    idxl = np.zeros(NCORES * ROWS_CORE, dtype=np.float32)
    idxl[dest] = (idx - np.repeat(np.arange(NWIN, dtype=np.int64) * WIN, counts)).astype(
        np.float32
    )
    wpad = np.ones(NCORES * ROWS_CORE, dtype=np.float32)
    wpad[dest] = w

    # device layout: per core, per group: [128 partitions, GROUP tiles, ...]
    xdev = (
        xpad.reshape(NCORES, NG, GROUP, P, D + 1)
        .transpose(0, 1, 3, 2, 4)
        .reshape(NCORES, NG * P, GROUP * (D + 1))
    )
    mask = np.zeros((NCORES * ROWS_CORE, WIN), dtype=np.uint8)
    mask[dest, idxl[dest].astype(np.int64)] = 1
    maskdev = (
        mask.reshape(NCORES, NG, GROUP, P, WIN)
        .transpose(0, 1, 3, 2, 4)
        .reshape(NCORES, NG * P, GROUP * WIN)
    )
    wdev = np.ascontiguousarray(wpad.reshape(NCORES, NT, P).transpose(0, 2, 1))
    return xdev, maskdev, wdev


def _ensure_ntff_hook():
    """The image's antenv package lacks axon_hooks; shim it so trace=True
    can register the ctypes NTFF hook from trn_agent_boot."""
    try:
        from antenv.axon_hooks import get_axon_ntff_profile_hook  # noqa: F401

        return True
    except ImportError:
        pass
    try:
        import types

        import antenv
        from trn_agent_boot.trn_boot import _ntff_profile_via_ctypes

        mod = types.ModuleType("antenv.axon_hooks")
        _hook = [None]
        mod.set_axon_ntff_profile_hook = lambda h: _hook.__setitem__(0, h)
        mod.get_axon_ntff_profile_hook = lambda: _hook[0]
        sys.modules["antenv.axon_hooks"] = mod
        antenv.axon_hooks = mod
        mod.set_axon_ntff_profile_hook(
            _ntff_profile_via_ctypes("/opt/axon/libaxon_pjrt.so")
        )
        return True
    except Exception as e:  # degrade to untraced run
        print(f"ntff hook install failed: {type(e).__name__}: {e}")
        return False


def kernel(x, index, weights, gate_w, gate_b, msg_w, msg_b, pow_p):
    global LAST_EXEC_NS, LAST_RESULTS

    x = np.ascontiguousarray(np.asarray(x, dtype=np.float32))
    idx = np.asarray(index).astype(np.int64).ravel()
    w = np.asarray(weights, dtype=np.float32).ravel()
    gate_w = np.asarray(gate_w, dtype=np.float32).reshape(D)
    gate_b = np.asarray(gate_b, dtype=np.float32).reshape(1)
    msg_w = np.ascontiguousarray(np.asarray(msg_w, dtype=np.float32))
    msg_b = np.asarray(msg_b, dtype=np.float32).reshape(D)
    pow_p = np.asarray(pow_p, dtype=np.float32).reshape(1)

    if not np.all(idx[1:] >= idx[:-1]):
        perm = np.argsort(idx, kind="stable")
        idx = idx[perm]
        x = x[perm]
        w = w[perm]

    xdev, maskdev, wdev = _shard_inputs(x, idx, w)

    gwrep = np.tile(gate_w[None, :], (P, GROUP)).astype(np.float32)
    msgbrep = np.tile(msg_b[None, :], (P, 1)).astype(np.float32)
    gatebrep = np.full((P, 1), gate_b[0], dtype=np.float32)
    prep = np.full((P, 1), pow_p[0], dtype=np.float32)
    nc = _build_module()
    from concourse.bass_utils import run_bass_kernel_spmd

    in_maps = []
    for c in range(NCORES):
        in_maps.append(
            {
                "xp": np.ascontiguousarray(xdev[c]),
                "maskg": np.ascontiguousarray(maskdev[c]),
                "wall": wdev[c],
                "gwrep": gwrep,
                "msgw": msg_w,
                "msgbrep": msgbrep,
                "gatebrep": gatebrep,
                "prep": prep,
            }
        )

    trace = bool(os.environ.get("KERNEL_TRACE"))
    if trace:
        trace = _ensure_ntff_hook()
    res = run_bass_kernel_spmd(
        nc, in_maps, core_ids=list(range(NCORES)), trace=trace
    )
    LAST_RESULTS = res
    LAST_EXEC_NS = res.exec_time_ns

    out = np.concatenate([res.results[c]["out"] for c in range(NCORES)], axis=0)
    return out.astype(np.float32)


def kernel_numpy(x, index, weights, gate_w, gate_b, msg_w, msg_b, pow_p):
    """Host-side mirror of the device algorithm (debug only)."""
    x = np.asarray(x, dtype=np.float32)
    idx = np.asarray(index).astype(np.int64).ravel()
    w = np.asarray(weights, dtype=np.float32).ravel()
    gate = x @ np.asarray(gate_w, dtype=np.float32).reshape(D, 1)
    gate = gate[:, 0] + np.asarray(gate_b).reshape(1)[0]
    g = np.exp(gate + np.asarray(pow_p).reshape(1)[0] * np.log(w))
    A = np.zeros((S, D), dtype=np.float64)
    den = np.zeros(S, dtype=np.float64)
    np.add.at(A, idx, g[:, None] * x)
    np.add.at(den, idx, g)
    out = (A @ np.asarray(msg_w, dtype=np.float64)) / (den[:, None] + EPS)
    out = out + (den / (den + EPS))[:, None] * np.asarray(msg_b).reshape(1, D)
    return out.astype(np.float32)



# revision 3
# speedup vs baseline: 2.3008x; 2.3008x over previous
"""AttentionPooling (segment softmax-pool) Trainium2 kernel — v2 (bf16).

out[s,:] = sum_n 1[idx[n]==s] * gnorm[n] * (x[n,:] @ msg_w + msg_b)
  gnorm[n] = w[n]^p * exp(gate[n]) / (denom[seg] + eps)   (max-sub skipped:
  exact after normalization, logits are O(6))

v2 reformulation (vs f32 v1): fold gate_w into x on the host
(x' = x*gw, msg_w' = msg_w/gw — diagonal reparametrization), and ship
p*ln(w) as column 0 of the x tile.  The whole per-row logit then
becomes ONE contiguous DVE reduce per 17-tile group:

  tile cols = [p*ln(w) | x*gw (128) | ones] (130 bf16)
  logit  = reduce_sum(cols 0:130)       (+1 from ones col: cancels in softmax,
                                         gate_b also cancels — never applied)
  gex    = exp(logit)                   (ACT, f32)
  G      = mask_u8 * gex                (1 broadcast TT on DVE + a few ACT
                                         copy-scale tiles for balance)
  psA   += G^T @ xs[:, 1:130]           (bf16 matmul, PSUM f32: [64, A|denom])

Phase 2 per 64-seg window: rcp = 1/(den+eps); Anorm = psA*rcp (ACT, also
yields coef = den*rcp in col 128); out = (AnormT^T @ msg_w') + coef*msg_b.

Sharding: index is sorted; host assigns 2048 contiguous segments per core,
32 windows x 64 segments, rows of each window padded to 34*128 = 4352.
"""

import os
import sys
import numpy as np

for _p in ("/opt/trn_rl_repo", "/root/.axon_site/_ro/trn_rl_repo"):
    if os.path.isdir(_p) and _p not in sys.path:
        sys.path.insert(0, _p)

P = 128
S = 16384
D = 128
NCORES = 8
WIN = 64                       # segments per PSUM window
NWIN = S // WIN                # 256 global windows
NWIN_CORE = NWIN // NCORES     # 32 per core
TPW = 34                       # 128-row tiles per window (padded)
GROUP = 17                     # tiles per DMA/logit super-group
GPW = TPW // GROUP             # 2 groups per window
NT = NWIN_CORE * TPW           # 1088 tiles per core
NG = NT // GROUP               # 64 groups per core
ROWS_CORE = NT * P             # 139264 padded rows per core
COLS = D + 2                   # [plw | x*gw .. | ones]
EPS = 1e-10

ACT_G_TILES = 6                # leading tiles of each group G-built on ACT

LAST_EXEC_NS = None
LAST_RESULTS = None

_module_cache = {}


def _build_module():
    if "nc" in _module_cache:
        return _module_cache["nc"]

    import concourse.bass as bass  # noqa: F401
    import concourse.tile as tile
    from concourse import bacc, mybir
    from concourse.masks import make_identity

    f32 = mybir.dt.float32
    bf16 = mybir.dt.bfloat16
    u8 = mybir.dt.uint8
    AX = mybir.AxisListType
    ALU = mybir.AluOpType
    ACTF = mybir.ActivationFunctionType

    nc = bacc.Bacc(
        "TRN2",
        target_bir_lowering=False,
        debug=False,
        enable_asserts=True,
        num_devices=NCORES,
    )

    xp = nc.dram_tensor("xp", [NG * P, GROUP * COLS], bf16, kind="ExternalInput")
    maskg = nc.dram_tensor("maskg", [NG * P, GROUP * WIN], u8, kind="ExternalInput")
    msgwp = nc.dram_tensor("msgwp", [D, D], f32, kind="ExternalInput")
    msgbrep = nc.dram_tensor("msgbrep", [P, D], f32, kind="ExternalInput")
    out = nc.dram_tensor("out", [NWIN_CORE * WIN, D], f32, kind="ExternalOutput")

    with tile.TileContext(nc) as tc:
        from contextlib import ExitStack

        with ExitStack() as ctx:
            const_pool = ctx.enter_context(tc.tile_pool(name="const", bufs=1))
            xs_pool = ctx.enter_context(tc.tile_pool(name="xs", bufs=6))
            grp_pool = ctx.enter_context(tc.tile_pool(name="grp", bufs=6))
            g_pool = ctx.enter_context(tc.tile_pool(name="gm", bufs=5))
            psA_pool = ctx.enter_context(tc.tile_pool(name="psA", bufs=4, space="PSUM"))
            ps2_pool = ctx.enter_context(tc.tile_pool(name="ps2", bufs=2, space="PSUM"))
            ph2_pool = ctx.enter_context(tc.tile_pool(name="ph2", bufs=3))

            msgw_t = const_pool.tile([D, D], f32)
            nc.sync.dma_start(msgw_t[:], msgwp[:, :])
            msgb_t = const_pool.tile([P, D], f32)
            nc.sync.dma_start(msgb_t[:], msgbrep[:, :])
            ident = const_pool.tile([P, P], f32)
            make_identity(nc, ident[:])

            # software pipeline: emit group g's logit chain before group
            # g-1's G-builds/matmuls so engines overlap across groups
            chains = {}

            def emit_chain(g):
                xs = xs_pool.tile([P, GROUP * COLS], bf16, tag="xs", name=f"xs{g}")
                nc.sync.dma_start(xs[:], xp[g * P : (g + 1) * P, :])
                mk = xs_pool.tile([P, GROUP * WIN], u8, tag="mk", name=f"mk{g}")
                nc.sync.dma_start(mk[:], maskg[g * P : (g + 1) * P, :])
                xs3 = xs[:].rearrange("p (j c) -> p j c", c=COLS)
                logit = grp_pool.tile([P, GROUP], f32, tag="logit", name=f"lg{g}")
                nc.vector.reduce_sum(out=logit[:], in_=xs3, axis=AX.X)
                gex = grp_pool.tile([P, GROUP], f32, tag="gex", name=f"gx{g}")
                nc.scalar.activation(out=gex[:], in_=logit[:], func=ACTF.Exp)
                chains[g] = (xs, mk, gex)

            def emit_gmm(g, psA):
                xs, mk, gex = chains.pop(g)
                Gt = g_pool.tile([P, GROUP * WIN], bf16, tag="G", name=f"G{g}")
                mk3 = mk[:].rearrange("p (j w) -> p j w", w=WIN)
                G3 = Gt[:].rearrange("p (j w) -> p j w", w=WIN)
                a = ACT_G_TILES
                for j in range(a):
                    nc.scalar.activation(
                        out=G3[:, j, :],
                        in_=mk3[:, j, :],
                        func=ACTF.Copy,
                        scale=gex[:, j : j + 1],
                    )
                if a < GROUP:
                    gexB = (
                        gex[:, a:GROUP]
                        .unsqueeze(2)
                        .broadcast_to([P, GROUP - a, WIN])
                    )
                    nc.vector.tensor_tensor(
                        out=G3[:, a:GROUP, :],
                        in0=mk3[:, a:GROUP, :],
                        in1=gexB,
                        op=ALU.mult,
                    )
                gi = g % GPW
                for j in range(GROUP):
                    t_in_win = gi * GROUP + j
                    nc.tensor.matmul(
                        out=psA[:],
                        lhsT=Gt[:, j * WIN : (j + 1) * WIN],
                        rhs=xs[:, j * COLS + 1 : j * COLS + COLS],
                        start=(t_in_win == 0),
                        stop=(t_in_win == TPW - 1),
                    )

            def emit_phase2(w, psA):
                deno = ph2_pool.tile([WIN, 1], f32, tag="deno", name=f"dn{w}")
                nc.vector.tensor_scalar_add(deno[:], psA[:, D : D + 1], EPS)
                rcp = ph2_pool.tile([WIN, 1], f32, tag="rcp", name=f"rc{w}")
                nc.vector.reciprocal(out=rcp[:], in_=deno[:])
                # Anorm = psA * rcp ; col 128 becomes coef = den/(den+eps)
                anorm = ph2_pool.tile([WIN, D + 1], f32, tag="anorm", name=f"an{w}")
                nc.scalar.activation(
                    out=anorm[:], in_=psA[:], func=ACTF.Copy, scale=rcp[:, 0:1]
                )
                psAT = ps2_pool.tile([P, WIN], f32, tag="AT", name=f"AT{w}")
                nc.tensor.transpose(
                    out=psAT[:], in_=anorm[:, 0:D], identity=ident[:WIN, :WIN]
                )
                sbAT = ph2_pool.tile([P, WIN], f32, tag="sbAT", name=f"sT{w}")
                nc.vector.tensor_copy(sbAT[:], psAT[:])
                ps2 = ps2_pool.tile([WIN, D], f32, tag="out2", name=f"o2{w}")
                nc.tensor.matmul(
                    out=ps2[:], lhsT=sbAT[:], rhs=msgw_t[:], start=True, stop=True
                )
                bterm = ph2_pool.tile([WIN, D], f32, tag="bterm", name=f"bt{w}")
                nc.scalar.activation(
                    out=bterm[:], in_=msgb_t[:WIN, :], func=ACTF.Copy,
                    scale=anorm[:, D : D + 1],
                )
                ofin = ph2_pool.tile([WIN, D], f32, tag="ofin", name=f"of{w}")
                nc.vector.tensor_add(ofin[:], ps2[:], bterm[:])
                nc.sync.dma_start(out[w * WIN : (w + 1) * WIN, :], ofin[:])

            psA_tiles = {}
            emit_chain(0)
            for g in range(NG):
                if g + 1 < NG:
                    emit_chain(g + 1)
                w = g // GPW
                if g % GPW == 0:
                    psA_tiles[w] = psA_pool.tile(
                        [WIN, D + 1], f32, tag="psA", name=f"psA{w}"
                    )
                emit_gmm(g, psA_tiles[w])
                if g % GPW == GPW - 1:
                    emit_phase2(w, psA_tiles.pop(w))

    nc.compile()
    _module_cache["nc"] = nc
    return nc


def _prep_inputs(x, idx, w, gw, pw):
    """Fold gate_w/pow into padded per-core bf16 layouts + u8 one-hot mask."""
    import ml_dtypes

    bf = ml_dtypes.bfloat16
    n = idx.shape[0]
    bounds = np.searchsorted(idx, np.arange(0, S + 1, WIN)).astype(np.int64)
    counts = np.diff(bounds)
    if counts.max() > TPW * P:
        raise RuntimeError(f"window overflow: {counts.max()} > {TPW * P}")

    dest = np.arange(n, dtype=np.int64) + np.repeat(
        np.arange(NWIN, dtype=np.int64) * (TPW * P) - bounds[:-1], counts
    )

    xpad = np.zeros((NCORES * ROWS_CORE, COLS), dtype=bf)
    xpad[dest, 0] = (pw * np.log(w.astype(np.float64))).astype(bf)
    xpad[dest, 1 : D + 1] = (x.astype(np.float64) * gw[None, :]).astype(bf)
    xpad[dest, D + 1] = np.float32(1.0)

    idxl = np.zeros(NCORES * ROWS_CORE, dtype=np.int64)
    idxl[dest] = idx - np.repeat(np.arange(NWIN, dtype=np.int64) * WIN, counts)
    mask = np.zeros((NCORES * ROWS_CORE, WIN), dtype=np.uint8)
    mask[dest, idxl[dest]] = 1

    xdev = (
        xpad.reshape(NCORES, NG, GROUP, P, COLS)
        .transpose(0, 1, 3, 2, 4)
        .reshape(NCORES, NG * P, GROUP * COLS)
    )
    maskdev = (
        mask.reshape(NCORES, NG, GROUP, P, WIN)
        .transpose(0, 1, 3, 2, 4)
        .reshape(NCORES, NG * P, GROUP * WIN)
    )
    return xdev, maskdev


def _ensure_ntff_hook():
    """The image's antenv package lacks axon_hooks; shim it so trace=True
    can register the ctypes NTFF hook from trn_agent_boot."""
    try:
        from antenv.axon_hooks import get_axon_ntff_profile_hook  # noqa: F401

        return True
    except ImportError:
        pass
    try:
        import types

        import antenv
        from trn_agent_boot.trn_boot import _ntff_profile_via_ctypes

        mod = types.ModuleType("antenv.axon_hooks")
        _hook = [None]
        mod.set_axon_ntff_profile_hook = lambda h: _hook.__setitem__(0, h)
        mod.get_axon_ntff_profile_hook = lambda: _hook[0]
        sys.modules["antenv.axon_hooks"] = mod
        antenv.axon_hooks = mod
        mod.set_axon_ntff_profile_hook(
            _ntff_profile_via_ctypes("/opt/axon/libaxon_pjrt.so")
        )
        return True
    except Exception as e:  # degrade to untraced run
        print(f"ntff hook install failed: {type(e).__name__}: {e}")
        return False


def kernel(x, index, weights, gate_w, gate_b, msg_w, msg_b, pow_p):
    global LAST_EXEC_NS, LAST_RESULTS

    x = np.ascontiguousarray(np.asarray(x, dtype=np.float32))
    idx = np.asarray(index).astype(np.int64).ravel()
    w = np.asarray(weights, dtype=np.float32).ravel()
    gate_w = np.asarray(gate_w, dtype=np.float64).reshape(D)
    msg_w = np.ascontiguousarray(np.asarray(msg_w, dtype=np.float64))
    msg_b = np.asarray(msg_b, dtype=np.float32).reshape(D)
    pw = float(np.asarray(pow_p, dtype=np.float64).reshape(1)[0])
    # gate_b shifts every logit equally -> cancels in the segment softmax.

    if not np.all(idx[1:] >= idx[:-1]):
        perm = np.argsort(idx, kind="stable")
        idx = idx[perm]
        x = x[perm]
        w = w[perm]

    xdev, maskdev = _prep_inputs(x, idx, w, gate_w, pw)

    msgwp = np.ascontiguousarray((msg_w / gate_w[:, None]).astype(np.float32))
    msgbrep = np.tile(msg_b[None, :], (P, 1)).astype(np.float32)

    nc = _build_module()
    from concourse.bass_utils import run_bass_kernel_spmd

    in_maps = []
    for c in range(NCORES):
        in_maps.append(
            {
                "xp": np.ascontiguousarray(xdev[c]),
                "maskg": np.ascontiguousarray(maskdev[c]),
                "msgwp": msgwp,
                "msgbrep": msgbrep,
            }
        )

    trace = bool(os.environ.get("KERNEL_TRACE"))
    if trace:
        trace = _ensure_ntff_hook()
    res = run_bass_kernel_spmd(
        nc, in_maps, core_ids=list(range(NCORES)), trace=trace
    )
    LAST_RESULTS = res
    LAST_EXEC_NS = res.exec_time_ns

    out = np.concatenate([res.results[c]["out"] for c in range(NCORES)], axis=0)
    return out.astype(np.float32)


def kernel_numpy(x, index, weights, gate_w, gate_b, msg_w, msg_b, pow_p):
    """Host-side mirror of the device algorithm (debug only)."""
    import ml_dtypes

    bf = ml_dtypes.bfloat16
    x = np.asarray(x, dtype=np.float64)
    idx = np.asarray(index).astype(np.int64).ravel()
    w = np.asarray(weights, dtype=np.float64).ravel()
    gw = np.asarray(gate_w, dtype=np.float64).reshape(D)
    pw = float(np.asarray(pow_p, dtype=np.float64).reshape(1)[0])
    xg = (x * gw[None, :]).astype(bf).astype(np.float32)
    plw = (pw * np.log(w)).astype(bf).astype(np.float32)
    logit = plw + xg.sum(axis=1, dtype=np.float32) + 1.0
    G = np.exp(logit).astype(np.float32).astype(bf).astype(np.float32)
    A = np.zeros((S, D), np.float32)
    den = np.zeros(S, np.float32)
    np.add.at(A, idx, G[:, None] * xg)
    np.add.at(den, idx, G)
    rcp = (1.0 / (den + EPS)).astype(np.float32)
    msgwp = (np.asarray(msg_w, np.float64) / gw[:, None]).astype(np.float32)
    out = (A * rcp[:, None]) @ msgwp + (den * rcp)[:, None] * np.asarray(
        msg_b, np.float32
    ).reshape(1, D)
    return out.astype(np.float32)


# revision 5
# speedup vs baseline: 2.5901x; 1.1257x over previous
"""AttentionPooling (segment softmax-pool) Trainium2 kernel — v3 (bf16).

out[s,:] = sum_n 1[idx[n]==s] * gnorm[n] * (x[n,:] @ msg_w + msg_b)
  gnorm[n] = w[n]^p * exp(gate[n]) / (denom[seg] + eps)   (max-sub skipped:
  exact after normalization, logits are O(6))

Reformulation: fold gate_w into x on the host (x' = x*gw,
msg_w' = msg_w/gw — diagonal reparametrization), ship p*ln(w) as
column 0.  The per-row logit then becomes ONE DVE reduce per window:

  tile row  = [p*ln(w) | x*gw (128) | ones] bf16 (130) ++ mask u8 (32)
  logit     = reduce_sum(bf16 cols)     (+1 const and gate_b cancel in softmax)
  gex       = exp(logit)                (ACT, f32)
  G         = mask_u8 * gex             (broadcast TT on DVE + a few ACT tiles)
  psA[h]   += G^T @ row[1:130]          (bf16 matmul, M=32, PSUM f32)

Windows are 32 segments; TWO windows stack in one [64, 129] PSUM tile
(partition offsets 0/32) so phase 2 runs once per pair:
  rcp = 1/(den+eps); Anorm = psA*rcp (ACT; col 128 -> coef = den*rcp);
  ps2 = AnormT^T @ msg_w' + coefT^T(K=1) @ msg_b;  out = copy(ps2).

Sharding: index is sorted; 2048 contiguous segments per core, 64
windows/core; per-window tile counts are the max over the 8 cores
(one SPMD program), derived from the actual index at first call.
"""

import os
import sys
import numpy as np

for _p in ("/opt/trn_rl_repo", "/root/.axon_site/_ro/trn_rl_repo"):
    if os.path.isdir(_p) and _p not in sys.path:
        sys.path.insert(0, _p)

P = 128
S = 16384
D = 128
NCORES = 8
WIN = 32                       # segments per PSUM half-window
NWIN = S // WIN                # 512 global windows
NWIN_CORE = NWIN // NCORES     # 64 per core
COLS = D + 2                   # [plw | x*gw .. | ones] bf16
RB = 2 * COLS + WIN            # merged row bytes per tile: 260 bf16 + 32 u8
CW = RB // 2                   # bf16 view cols per tile (146)
EPS = 1e-10

ACT_G_TILES = 3                # leading tiles per window G-built on ACT

LAST_EXEC_NS = None
LAST_RESULTS = None

_module_cache = {}


def _build_module(t_slots):
    key = ("v3", t_slots, ACT_G_TILES)
    if key in _module_cache:
        return _module_cache[key]

    import concourse.bass as bass  # noqa: F401
    import concourse.tile as tile
    from concourse import bacc, mybir
    from concourse.masks import make_identity

    f32 = mybir.dt.float32
    bf16 = mybir.dt.bfloat16
    u8 = mybir.dt.uint8
    AX = mybir.AxisListType
    ALU = mybir.AluOpType
    ACTF = mybir.ActivationFunctionType

    NT = sum(t_slots)

    nc = bacc.Bacc(
        "TRN2",
        target_bir_lowering=False,
        debug=False,
        enable_asserts=True,
        num_devices=NCORES,
    )

    xm = nc.dram_tensor("xm", [NWIN_CORE * P, max(t_slots) * RB], u8,
                        kind="ExternalInput")
    # xm row w*P+p holds t_slots[w]*RB valid bytes (rest padding)
    msgwp = nc.dram_tensor("msgwp", [D, D], f32, kind="ExternalInput")
    msgbrow = nc.dram_tensor("msgbrow", [1, D], f32, kind="ExternalInput")
    out = nc.dram_tensor("out", [NWIN_CORE * WIN, D], f32, kind="ExternalOutput")

    with tile.TileContext(nc) as tc:
        from contextlib import ExitStack

        with ExitStack() as ctx:
            const_pool = ctx.enter_context(tc.tile_pool(name="const", bufs=1))
            xs_pool = ctx.enter_context(tc.tile_pool(name="xs", bufs=6))
            grp_pool = ctx.enter_context(tc.tile_pool(name="grp", bufs=6))
            g_pool = ctx.enter_context(tc.tile_pool(name="gm", bufs=5))
            psA_pool = ctx.enter_context(tc.tile_pool(name="psA", bufs=2, space="PSUM"))
            ps2_pool = ctx.enter_context(tc.tile_pool(name="ps2", bufs=2, space="PSUM"))
            ph2_pool = ctx.enter_context(tc.tile_pool(name="ph2", bufs=3))

            msgw_t = const_pool.tile([D, D], f32)
            nc.sync.dma_start(msgw_t[:], msgwp[:, :])
            msgb_t = const_pool.tile([1, D], f32)
            nc.sync.dma_start(msgb_t[:], msgbrow[:, :])
            ident = const_pool.tile([P, P], f32)
            make_identity(nc, ident[:])

            chains = {}

            def emit_chain(w):
                T = t_slots[w]
                xt = xs_pool.tile([P, T * RB], u8, tag="xs", name=f"xs{w}")
                nc.sync.dma_start(xt[:], xm[w * P : (w + 1) * P, 0 : T * RB])
                xb = xt[:].bitcast(bf16)                       # [P, T*CW]
                xb3 = xb.rearrange("p (j c) -> p j c", c=CW)
                logit = grp_pool.tile([P, T], f32, tag="logit", name=f"lg{w}")
                nc.vector.reduce_sum(out=logit[:], in_=xb3[:, :, 0:COLS], axis=AX.X)
                gex = grp_pool.tile([P, T], f32, tag="gex", name=f"gx{w}")
                nc.scalar.activation(out=gex[:], in_=logit[:], func=ACTF.Exp)
                chains[w] = (xt, gex)

            def emit_gmm(w, psA, half):
                T = t_slots[w]
                xt, gex = chains.pop(w)
                xb = xt[:].bitcast(bf16)
                mk3 = xt[:].rearrange("p (j b) -> p j b", b=RB)
                Gt = g_pool.tile([P, T * WIN], bf16, tag="G", name=f"G{w}")
                G3 = Gt[:].rearrange("p (j s) -> p j s", s=WIN)
                a = min(ACT_G_TILES, T)
                for j in range(a):
                    nc.scalar.activation(
                        out=G3[:, j, :],
                        in_=mk3[:, j, 2 * COLS : RB],
                        func=ACTF.Copy,
                        scale=gex[:, j : j + 1],
                    )
                if a < T:
                    gexB = gex[:, a:T].unsqueeze(2).broadcast_to([P, T - a, WIN])
                    nc.vector.tensor_tensor(
                        out=G3[:, a:T, :],
                        in0=mk3[:, a:T, 2 * COLS : RB],
                        in1=gexB,
                        op=ALU.mult,
                    )
                for j in range(T):
                    nc.tensor.matmul(
                        out=psA[half * WIN : (half + 1) * WIN, :],
                        lhsT=Gt[:, j * WIN : (j + 1) * WIN],
                        rhs=xb[:, j * CW + 1 : j * CW + COLS],
                        start=(j == 0),
                        stop=(j == T - 1),
                    )

            def emit_phase2(pair, psA):
                deno = ph2_pool.tile([2 * WIN, 1], f32, tag="deno", name=f"dn{pair}")
                nc.vector.tensor_scalar_add(deno[:], psA[:, D : D + 1], EPS)
                rcp = ph2_pool.tile([2 * WIN, 1], f32, tag="rcp", name=f"rc{pair}")
                nc.vector.reciprocal(out=rcp[:], in_=deno[:])
                # Anorm = psA * rcp ; col 128 becomes coef = den/(den+eps)
                anorm = ph2_pool.tile([2 * WIN, D + 1], f32, tag="anorm",
                                      name=f"an{pair}")
                nc.scalar.activation(
                    out=anorm[:], in_=psA[:], func=ACTF.Copy, scale=rcp[:, 0:1]
                )
                psAT = ps2_pool.tile([P, 2 * WIN], f32, tag="AT", name=f"AT{pair}")
                nc.tensor.transpose(
                    out=psAT[:], in_=anorm[:, 0:D],
                    identity=ident[: 2 * WIN, : 2 * WIN],
                )
                sbAT = ph2_pool.tile([P, 2 * WIN], f32, tag="sbAT", name=f"sT{pair}")
                nc.vector.tensor_copy(sbAT[:], psAT[:])
                psCT = ps2_pool.tile([1, 2 * WIN], f32, tag="CT", name=f"CT{pair}")
                nc.tensor.transpose(
                    out=psCT[:], in_=anorm[:, D : D + 1],
                    identity=ident[: 2 * WIN, : 2 * WIN],
                )
                sbCT = ph2_pool.tile([1, 2 * WIN], f32, tag="sbCT", name=f"sC{pair}")
                nc.vector.tensor_copy(sbCT[:], psCT[:])
                ps2 = ps2_pool.tile([2 * WIN, D], f32, tag="out2", name=f"o2{pair}")
                nc.tensor.matmul(
                    out=ps2[:], lhsT=sbAT[:], rhs=msgw_t[:], start=True, stop=False
                )
                nc.tensor.matmul(
                    out=ps2[:], lhsT=sbCT[:], rhs=msgb_t[:], start=False, stop=True
                )
                ofin = ph2_pool.tile([2 * WIN, D], f32, tag="ofin", name=f"of{pair}")
                nc.scalar.activation(out=ofin[:], in_=ps2[:], func=ACTF.Copy)
                nc.sync.dma_start(
                    out[pair * 2 * WIN : (pair + 1) * 2 * WIN, :], ofin[:]
                )

            psA_tiles = {}
            emit_chain(0)
            for w in range(NWIN_CORE):
                if w + 1 < NWIN_CORE:
                    emit_chain(w + 1)
                pair = w // 2
                half = w % 2
                if half == 0:
                    psA_tiles[pair] = psA_pool.tile(
                        [2 * WIN, D + 1], f32, tag="psA", name=f"psA{pair}"
                    )
                emit_gmm(w, psA_tiles[pair], half)
                if half == 1:
                    emit_phase2(pair, psA_tiles.pop(pair))

    nc.compile()
    _module_cache[key] = nc
    return nc


def _prep_inputs(x, idx, w, gw, pw):
    """Fold gate_w/pow into merged per-core [x|mask] device rows."""
    import ml_dtypes

    bf = ml_dtypes.bfloat16
    n = idx.shape[0]
    bounds = np.searchsorted(idx, np.arange(0, S + 1, WIN)).astype(np.int64)
    counts = np.diff(bounds)                          # [NWIN] rows per window
    t_real = (counts + P - 1) // P                    # tiles per global window
    # one SPMD program: per-window-slot tile count = max over the 8 cores
    t_slots = tuple(int(v) for v in t_real.reshape(NCORES, NWIN_CORE).max(axis=0))
    tpw = np.tile(np.asarray(t_slots, np.int64), NCORES)   # padded tiles per window
    row_cap = tpw * P
    wstart = np.concatenate([[0], np.cumsum(row_cap)])[:-1]  # padded row offset

    dest = np.arange(n, dtype=np.int64) + np.repeat(wstart - bounds[:-1], counts)

    total_rows = int(row_cap.sum())
    xpad = np.zeros((total_rows, COLS), dtype=bf)
    xpad[dest, 0] = (pw * np.log(w.astype(np.float64))).astype(bf)
    xpad[dest, 1 : D + 1] = (x.astype(np.float64) * gw[None, :]).astype(bf)
    xpad[dest, D + 1] = np.float32(1.0)

    segl = idx - np.repeat(np.arange(NWIN, dtype=np.int64) * WIN, counts)
    mask = np.zeros((total_rows, WIN), dtype=np.uint8)
    mask[dest, segl] = 1

    # merged rows: [130 bf16 | 32 u8] per tile, gathered to device layout
    merged = np.concatenate([xpad.view(np.uint8), mask], axis=1)  # [rows, RB]
    tmax = max(t_slots)
    xdev = np.zeros((NCORES, NWIN_CORE * P, tmax * RB), dtype=np.uint8)
    rows_per_core = total_rows // NCORES
    for c in range(NCORES):
        off = c * rows_per_core
        for wi in range(NWIN_CORE):
            T = t_slots[wi]
            blk = merged[off : off + T * P].reshape(T, P, RB)
            xdev[c, wi * P : (wi + 1) * P, 0 : T * RB] = (
                blk.transpose(1, 0, 2).reshape(P, T * RB)
            )
            off += T * P
    return xdev, t_slots


def _ensure_ntff_hook():
    """The image's antenv package lacks axon_hooks; shim it so trace=True
    can register the ctypes NTFF hook from trn_agent_boot."""
    try:
        from antenv.axon_hooks import get_axon_ntff_profile_hook  # noqa: F401

        return True
    except ImportError:
        pass
    try:
        import types

        import antenv
        from trn_agent_boot.trn_boot import _ntff_profile_via_ctypes

        mod = types.ModuleType("antenv.axon_hooks")
        _hook = [None]
        mod.set_axon_ntff_profile_hook = lambda h: _hook.__setitem__(0, h)
        mod.get_axon_ntff_profile_hook = lambda: _hook[0]
        sys.modules["antenv.axon_hooks"] = mod
        antenv.axon_hooks = mod
        mod.set_axon_ntff_profile_hook(
            _ntff_profile_via_ctypes("/opt/axon/libaxon_pjrt.so")
        )
        return True
    except Exception as e:  # degrade to untraced run
        print(f"ntff hook install failed: {type(e).__name__}: {e}")
        return False


def kernel(x, index, weights, gate_w, gate_b, msg_w, msg_b, pow_p):
    global LAST_EXEC_NS, LAST_RESULTS

    x = np.ascontiguousarray(np.asarray(x, dtype=np.float32))
    idx = np.asarray(index).astype(np.int64).ravel()
    w = np.asarray(weights, dtype=np.float32).ravel()
    gate_w = np.asarray(gate_w, dtype=np.float64).reshape(D)
    msg_w = np.ascontiguousarray(np.asarray(msg_w, dtype=np.float64))
    msg_b = np.asarray(msg_b, dtype=np.float32).reshape(D)
    pw = float(np.asarray(pow_p, dtype=np.float64).reshape(1)[0])
    # gate_b shifts every logit equally -> cancels in the segment softmax.

    if not np.all(idx[1:] >= idx[:-1]):
        perm = np.argsort(idx, kind="stable")
        idx = idx[perm]
        x = x[perm]
        w = w[perm]

    xdev, t_slots = _prep_inputs(x, idx, w, gate_w, pw)

    msgwp = np.ascontiguousarray((msg_w / gate_w[:, None]).astype(np.float32))
    msgbrow = np.asarray(msg_b, np.float32).reshape(1, D)

    nc = _build_module(t_slots)
    from concourse.bass_utils import run_bass_kernel_spmd

    in_maps = []
    for c in range(NCORES):
        in_maps.append(
            {
                "xm": np.ascontiguousarray(xdev[c]),
                "msgwp": msgwp,
                "msgbrow": msgbrow,
            }
        )

    trace = bool(os.environ.get("KERNEL_TRACE"))
    if trace:
        trace = _ensure_ntff_hook()
    res = run_bass_kernel_spmd(
        nc, in_maps, core_ids=list(range(NCORES)), trace=trace
    )
    LAST_RESULTS = res
    LAST_EXEC_NS = res.exec_time_ns

    out = np.concatenate([res.results[c]["out"] for c in range(NCORES)], axis=0)
    return out.astype(np.float32)


def kernel_numpy(x, index, weights, gate_w, gate_b, msg_w, msg_b, pow_p):
    """Host-side mirror of the device algorithm (debug only)."""
    import ml_dtypes

    bf = ml_dtypes.bfloat16
    x = np.asarray(x, dtype=np.float64)
    idx = np.asarray(index).astype(np.int64).ravel()
    w = np.asarray(weights, dtype=np.float64).ravel()
    gw = np.asarray(gate_w, dtype=np.float64).reshape(D)
    pw = float(np.asarray(pow_p, dtype=np.float64).reshape(1)[0])
    xg = (x * gw[None, :]).astype(bf).astype(np.float32)
    plw = (pw * np.log(w)).astype(bf).astype(np.float32)
    logit = plw + xg.sum(axis=1, dtype=np.float32) + 1.0
    G = np.exp(logit).astype(np.float32).astype(bf).astype(np.float32)
    A = np.zeros((S, D), np.float32)
    den = np.zeros(S, np.float32)
    np.add.at(A, idx, G[:, None] * xg)
    np.add.at(den, idx, G)
    rcp = (1.0 / (den + EPS)).astype(np.float32)
    msgwp = (np.asarray(msg_w, np.float64) / gw[:, None]).astype(np.float32)
    out = (A * rcp[:, None]) @ msgwp + (den * rcp)[:, None] * np.asarray(
        msg_b, np.float32
    ).reshape(1, D)
    return out.astype(np.float32)


# revision 7
# speedup vs baseline: 3.1181x; 1.2039x over previous
"""AttentionPooling (segment softmax-pool) Trainium2 kernel — v4 (bf16).

out[s,:] = sum_n 1[idx[n]==s] * gnorm[n] * (x[n,:] @ msg_w + msg_b)
  gnorm[n] = w[n]^p * exp(gate[n]) / (denom[seg] + eps)   (max-sub skipped:
  exact after normalization, logits are O(6))

Reformulation: fold gate_w into x on the host (x' = x*gw,
msg_w' = msg_w/gw — diagonal reparametrization), ship p*ln(w) as an
extra column.  Per-row logit per window (T tiles of 128 rows):

  tile row = [x*gw (128) | ones | p*ln(w) | pad pad] bf16 ++ mask u8 (32)
  h        = x'[0:64] + x'[64:128]      (TT, bf16 2x mode)
  logit_x  = reduce_sum(h)              (DVE 1x, half width)
  logit    = (plw + 1) + logit_x        (STT; +1/gate_b cancel in softmax)
  tail tile's logit via ACT accum (Copy over cols 0:130) to balance engines
  gex      = exp(logit)                 (ACT, f32)
  G        = mask_u8 * gex              (broadcast TT on DVE + a few ACT tiles)
  psA[h]  += G^T @ row[0:129]           (bf16 matmul, M=32, PSUM f32)

Windows are 32 segments; TWO windows stack in one [64, 129] PSUM tile
(partition offsets 0/32) so phase 2 runs once per pair (bf16):
  rcp = 1/(den+eps); Anorm = psA*rcp (ACT; col 128 -> coef = den*rcp);
  ps2 = AnormT^T @ msg_w' + coefT^T(K=1) @ msg_b;  out = copy(ps2).

Sharding: index is sorted; 2048 contiguous segments per core, 64
windows/core; per-window tile counts are the max over the 8 cores
(one SPMD program), derived from the actual index at first call.
"""

import os
import sys
import numpy as np

for _p in ("/opt/trn_rl_repo", "/root/.axon_site/_ro/trn_rl_repo"):
    if os.path.isdir(_p) and _p not in sys.path:
        sys.path.insert(0, _p)

P = 128
S = 16384
D = 128
NCORES = 8
WIN = 32                       # segments per PSUM half-window
NWIN = S // WIN                # 512 global windows
NWIN_CORE = NWIN // NCORES     # 64 per core
CONE = D                       # ones column (kept adjacent to x' for the matmul)
CPLW = D + 1                   # plw column
NBF = D + 4                    # bf16 cols per tile (132: 2 pad for 4B align)
RB = 2 * NBF + WIN             # merged row bytes per tile: 264 bf16 + 32 u8
CW = RB // 2                   # bf16 view cols per tile (148)
EPS = 1e-10

ACT_G_TILES = 3                # leading tiles per window G-built on ACT
K_TAIL = 1                     # trailing tiles per window logit-reduced on ACT

LAST_EXEC_NS = None
LAST_RESULTS = None

_module_cache = {}


def _build_module(t_slots):
    key = ("v4", t_slots, ACT_G_TILES, K_TAIL)
    if key in _module_cache:
        return _module_cache[key]

    import concourse.bass as bass  # noqa: F401
    import concourse.tile as tile
    from concourse import bacc, mybir
    from concourse.masks import make_identity

    f32 = mybir.dt.float32
    bf16 = mybir.dt.bfloat16
    u8 = mybir.dt.uint8
    AX = mybir.AxisListType
    ALU = mybir.AluOpType
    ACTF = mybir.ActivationFunctionType

    tmax = max(t_slots)

    nc = bacc.Bacc(
        "TRN2",
        target_bir_lowering=False,
        debug=False,
        enable_asserts=True,
        num_devices=NCORES,
    )

    xm = nc.dram_tensor("xm", [NWIN_CORE * P, tmax * RB], u8,
                        kind="ExternalInput")
    msgwp = nc.dram_tensor("msgwp", [D, D], bf16, kind="ExternalInput")
    msgbrow = nc.dram_tensor("msgbrow", [1, D], bf16, kind="ExternalInput")
    out = nc.dram_tensor("out", [NWIN_CORE * WIN, D], f32, kind="ExternalOutput")

    with tile.TileContext(nc) as tc:
        from contextlib import ExitStack

        with ExitStack() as ctx:
            const_pool = ctx.enter_context(tc.tile_pool(name="const", bufs=1))
            xs_pool = ctx.enter_context(tc.tile_pool(name="xs", bufs=6))
            grp_pool = ctx.enter_context(tc.tile_pool(name="grp", bufs=8))
            g_pool = ctx.enter_context(tc.tile_pool(name="gm", bufs=4))
            psA_pool = ctx.enter_context(tc.tile_pool(name="psA", bufs=2, space="PSUM"))
            ps2_pool = ctx.enter_context(tc.tile_pool(name="ps2", bufs=2, space="PSUM"))
            ph2_pool = ctx.enter_context(tc.tile_pool(name="ph2", bufs=3))

            msgw_t = const_pool.tile([D, D], bf16)
            nc.sync.dma_start(msgw_t[:], msgwp[:, :])
            msgb_t = const_pool.tile([1, D], bf16)
            nc.sync.dma_start(msgb_t[:], msgbrow[:, :])
            ident_bf = const_pool.tile([2 * WIN, 2 * WIN], bf16)
            make_identity(nc, ident_bf[:])

            chains = {}
            gmats = {}

            def emit_chain(w):
                T = t_slots[w]
                nT = T - K_TAIL
                xt = xs_pool.tile([P, T * RB], u8, tag="xs", name=f"xs{w}")
                nc.sync.dma_start(xt[:], xm[w * P : (w + 1) * P, 0 : T * RB])
                xb = xt[:].bitcast(bf16)                       # [P, T*CW]
                xb3 = xb.rearrange("p (j c) -> p j c", c=CW)
                logit = grp_pool.tile([P, T], f32, tag="logit", name=f"lg{w}")
                # DVE path: pair-fold x' at 2x, then half-width 1x reduce
                h = grp_pool.tile([P, nT * 64], bf16, tag="h", name=f"h{w}")
                h3 = h[:].rearrange("p (j c) -> p j c", c=64)
                nc.vector.tensor_tensor(
                    out=h3,
                    in0=xb3[:, 0:nT, 0:64],
                    in1=xb3[:, 0:nT, 64:D],
                    op=ALU.add,
                )
                lx = grp_pool.tile([P, nT], f32, tag="lx", name=f"lx{w}")
                nc.vector.reduce_sum(out=lx[:], in_=h3, axis=AX.X)
                nc.vector.scalar_tensor_tensor(
                    out=logit[:, 0:nT],
                    in0=xb3[:, 0:nT, CPLW],
                    scalar=1.0,
                    in1=lx[:],
                    op0=ALU.add,
                    op1=ALU.add,
                )
                # ACT path for tail tiles: accum over [x'|plw|ones] = logit+1
                for j in range(nT, T):
                    junk = grp_pool.tile([P, CPLW + 1], bf16, tag="junk",
                                         name=f"jk{w}_{j}")
                    nc.scalar.activation(
                        out=junk[:],
                        in_=xb[:, j * CW : j * CW + CPLW + 1],
                        func=ACTF.Copy,
                        accum_out=logit[:, j : j + 1],
                    )
                gex = grp_pool.tile([P, T], f32, tag="gex", name=f"gx{w}")
                nc.scalar.activation(out=gex[:], in_=logit[:], func=ACTF.Exp)
                chains[w] = (xt, gex)

            def emit_G(w):
                T = t_slots[w]
                xt, gex = chains.pop(w)
                mk3 = xt[:].rearrange("p (j b) -> p j b", b=RB)
                Gt = g_pool.tile([P, T * WIN], bf16, tag="G", name=f"G{w}")
                G3 = Gt[:].rearrange("p (j s) -> p j s", s=WIN)
                a = min(ACT_G_TILES, T)
                for j in range(a):
                    nc.scalar.activation(
                        out=G3[:, j, :],
                        in_=mk3[:, j, 2 * NBF : RB],
                        func=ACTF.Copy,
                        scale=gex[:, j : j + 1],
                    )
                if a < T:
                    gexB = gex[:, a:T].unsqueeze(2).broadcast_to([P, T - a, WIN])
                    nc.vector.tensor_tensor(
                        out=G3[:, a:T, :],
                        in0=mk3[:, a:T, 2 * NBF : RB],
                        in1=gexB,
                        op=ALU.mult,
                    )
                gmats[w] = (xt, Gt)

            def emit_mms(w, psA, half):
                T = t_slots[w]
                xt, Gt = gmats.pop(w)
                xb = xt[:].bitcast(bf16)
                for j in range(T):
                    nc.tensor.matmul(
                        out=psA[half * WIN : (half + 1) * WIN, :],
                        lhsT=Gt[:, j * WIN : (j + 1) * WIN],
                        rhs=xb[:, j * CW : j * CW + D + 1],
                        start=(j == 0),
                        stop=(j == T - 1),
                    )

            def emit_phase2(pair, psA):
                deno = ph2_pool.tile([2 * WIN, 1], f32, tag="deno", name=f"dn{pair}")
                nc.vector.tensor_scalar_add(deno[:], psA[:, D : D + 1], EPS)
                rcp = ph2_pool.tile([2 * WIN, 1], f32, tag="rcp", name=f"rc{pair}")
                nc.vector.reciprocal(out=rcp[:], in_=deno[:])
                # Anorm = psA * rcp ; col 128 becomes coef = den/(den+eps)
                anorm = ph2_pool.tile([2 * WIN, D + 1], bf16, tag="anorm",
                                      name=f"an{pair}")
                nc.scalar.activation(
                    out=anorm[:], in_=psA[:], func=ACTF.Copy, scale=rcp[:, 0:1]
                )
                psAT = ps2_pool.tile([P, 2 * WIN], bf16, tag="AT", name=f"AT{pair}")
                nc.tensor.transpose(
                    out=psAT[:], in_=anorm[:, 0:D], identity=ident_bf[:]
                )
                sbAT = ph2_pool.tile([P, 2 * WIN], bf16, tag="sbAT", name=f"sT{pair}")
                nc.scalar.activation(out=sbAT[:], in_=psAT[:], func=ACTF.Copy)
                psCT = ps2_pool.tile([1, 2 * WIN], bf16, tag="CT", name=f"CT{pair}")
                nc.tensor.transpose(
                    out=psCT[:], in_=anorm[:, D : D + 1], identity=ident_bf[:]
                )
                sbCT = ph2_pool.tile([1, 2 * WIN], bf16, tag="sbCT", name=f"sC{pair}")
                nc.vector.tensor_copy(sbCT[:], psCT[:])
                ps2 = ps2_pool.tile([2 * WIN, D], f32, tag="out2", name=f"o2{pair}")
                nc.tensor.matmul(
                    out=ps2[:], lhsT=sbAT[:], rhs=msgw_t[:], start=True, stop=False
                )
                nc.tensor.matmul(
                    out=ps2[:], lhsT=sbCT[:], rhs=msgb_t[:], start=False, stop=True
                )
                ofin = ph2_pool.tile([2 * WIN, D], f32, tag="ofin", name=f"of{pair}")
                nc.scalar.activation(out=ofin[:], in_=ps2[:], func=ACTF.Copy)
                nc.sync.dma_start(
                    out[pair * 2 * WIN : (pair + 1) * 2 * WIN, :], ofin[:]
                )

            psA_tiles = {}
            emit_chain(0)
            emit_chain(1)
            emit_G(0)
            for w in range(NWIN_CORE):
                if w + 2 < NWIN_CORE:
                    emit_chain(w + 2)
                if w + 1 < NWIN_CORE:
                    emit_G(w + 1)
                pair = w // 2
                half = w % 2
                if half == 0:
                    psA_tiles[pair] = psA_pool.tile(
                        [2 * WIN, D + 1], f32, tag="psA", name=f"psA{pair}"
                    )
                emit_mms(w, psA_tiles[pair], half)
                if half == 1:
                    emit_phase2(pair, psA_tiles.pop(pair))

    nc.compile()
    _module_cache[key] = nc
    return nc


def _prep_inputs(x, idx, w, gw, pw):
    """Fold gate_w/pow into merged per-core [x|mask] device rows."""
    import ml_dtypes

    bf = ml_dtypes.bfloat16
    n = idx.shape[0]
    bounds = np.searchsorted(idx, np.arange(0, S + 1, WIN)).astype(np.int64)
    counts = np.diff(bounds)                          # [NWIN] rows per window
    t_real = (counts + P - 1) // P                    # tiles per global window
    # one SPMD program: per-window-slot tile count = max over the 8 cores
    t_slots = tuple(int(v) for v in t_real.reshape(NCORES, NWIN_CORE).max(axis=0))
    tpw = np.tile(np.asarray(t_slots, np.int64), NCORES)   # padded tiles per window
    row_cap = tpw * P
    wstart = np.concatenate([[0], np.cumsum(row_cap)])[:-1]  # padded row offset

    dest = np.arange(n, dtype=np.int64) + np.repeat(wstart - bounds[:-1], counts)

    total_rows = int(row_cap.sum())
    xpad = np.zeros((total_rows, NBF), dtype=bf)
    xpad[dest, 0:D] = (x.astype(np.float64) * gw[None, :]).astype(bf)
    xpad[dest, CPLW] = (pw * np.log(w.astype(np.float64))).astype(bf)
    xpad[dest, CONE] = np.float32(1.0)

    segl = idx - np.repeat(np.arange(NWIN, dtype=np.int64) * WIN, counts)
    mask = np.zeros((total_rows, WIN), dtype=np.uint8)
    mask[dest, segl] = 1

    # merged rows: [132 bf16 | 32 u8] per tile, gathered to device layout
    merged = np.concatenate([xpad.view(np.uint8), mask], axis=1)  # [rows, RB]
    tmax = max(t_slots)
    xdev = np.zeros((NCORES, NWIN_CORE * P, tmax * RB), dtype=np.uint8)
    rows_per_core = total_rows // NCORES
    for c in range(NCORES):
        off = c * rows_per_core
        for wi in range(NWIN_CORE):
            T = t_slots[wi]
            blk = merged[off : off + T * P].reshape(T, P, RB)
            xdev[c, wi * P : (wi + 1) * P, 0 : T * RB] = (
                blk.transpose(1, 0, 2).reshape(P, T * RB)
            )
            off += T * P
    return xdev, t_slots


def _ensure_ntff_hook():
    """The image's antenv package lacks axon_hooks; shim it so trace=True
    can register the ctypes NTFF hook from trn_agent_boot."""
    try:
        from antenv.axon_hooks import get_axon_ntff_profile_hook  # noqa: F401

        return True
    except ImportError:
        pass
    try:
        import types

        import antenv
        from trn_agent_boot.trn_boot import _ntff_profile_via_ctypes

        mod = types.ModuleType("antenv.axon_hooks")
        _hook = [None]
        mod.set_axon_ntff_profile_hook = lambda h: _hook.__setitem__(0, h)
        mod.get_axon_ntff_profile_hook = lambda: _hook[0]
        sys.modules["antenv.axon_hooks"] = mod
        antenv.axon_hooks = mod
        mod.set_axon_ntff_profile_hook(
            _ntff_profile_via_ctypes("/opt/axon/libaxon_pjrt.so")
        )
        return True
    except Exception as e:  # degrade to untraced run
        print(f"ntff hook install failed: {type(e).__name__}: {e}")
        return False


def kernel(x, index, weights, gate_w, gate_b, msg_w, msg_b, pow_p):
    global LAST_EXEC_NS, LAST_RESULTS

    x = np.ascontiguousarray(np.asarray(x, dtype=np.float32))
    idx = np.asarray(index).astype(np.int64).ravel()
    w = np.asarray(weights, dtype=np.float32).ravel()
    gate_w = np.asarray(gate_w, dtype=np.float64).reshape(D)
    msg_w = np.ascontiguousarray(np.asarray(msg_w, dtype=np.float64))
    msg_b = np.asarray(msg_b, dtype=np.float32).reshape(D)
    pw = float(np.asarray(pow_p, dtype=np.float64).reshape(1)[0])
    # gate_b shifts every logit equally -> cancels in the segment softmax.

    if not np.all(idx[1:] >= idx[:-1]):
        perm = np.argsort(idx, kind="stable")
        idx = idx[perm]
        x = x[perm]
        w = w[perm]

    xdev, t_slots = _prep_inputs(x, idx, w, gate_w, pw)

    import ml_dtypes

    bf = ml_dtypes.bfloat16
    msgwp = np.ascontiguousarray((msg_w / gate_w[:, None]).astype(bf))
    msgbrow = np.asarray(msg_b, np.float32).reshape(1, D).astype(bf)

    nc = _build_module(t_slots)
    from concourse.bass_utils import run_bass_kernel_spmd

    in_maps = []
    for c in range(NCORES):
        in_maps.append(
            {
                "xm": np.ascontiguousarray(xdev[c]),
                "msgwp": msgwp,
                "msgbrow": msgbrow,
            }
        )

    trace = bool(os.environ.get("KERNEL_TRACE"))
    if trace:
        trace = _ensure_ntff_hook()
    res = run_bass_kernel_spmd(
        nc, in_maps, core_ids=list(range(NCORES)), trace=trace
    )
    LAST_RESULTS = res
    LAST_EXEC_NS = res.exec_time_ns

    out = np.concatenate([res.results[c]["out"] for c in range(NCORES)], axis=0)
    return out.astype(np.float32)


def kernel_numpy(x, index, weights, gate_w, gate_b, msg_w, msg_b, pow_p):
    """Host-side mirror of the device algorithm (debug only)."""
    import ml_dtypes

    bf = ml_dtypes.bfloat16
    x = np.asarray(x, dtype=np.float64)
    idx = np.asarray(index).astype(np.int64).ravel()
    w = np.asarray(weights, dtype=np.float64).ravel()
    gw = np.asarray(gate_w, dtype=np.float64).reshape(D)
    pw = float(np.asarray(pow_p, dtype=np.float64).reshape(1)[0])
    xg = (x * gw[None, :]).astype(bf).astype(np.float32)
    plw = (pw * np.log(w)).astype(bf).astype(np.float32)
    h = (xg[:, 0:64] + xg[:, 64:128]).astype(bf).astype(np.float32)
    logit = plw + h.sum(axis=1, dtype=np.float32) + 1.0
    G = np.exp(logit).astype(np.float32).astype(bf).astype(np.float32)
    A = np.zeros((S, D), np.float32)
    den = np.zeros(S, np.float32)
    np.add.at(A, idx, G[:, None] * xg)
    np.add.at(den, idx, G)
    rcp = (1.0 / (den + EPS)).astype(np.float32)
    msgwp = (np.asarray(msg_w, np.float64) / gw[:, None]).astype(bf).astype(
        np.float32
    )
    anorm = (A * rcp[:, None]).astype(bf).astype(np.float32)
    coef = (den * rcp).astype(bf).astype(np.float32)
    out = anorm @ msgwp + coef[:, None] * np.asarray(msg_b, np.float32).reshape(
        1, D
    ).astype(np.float32)
    return out.astype(np.float32)
